# revision 3
# baseline (speedup 1.0000x reference)
"""Trainium2 Bass kernel for nn_DocumentHead (retrieval head MLP).

Math (per batch row):
    align = <v_claim, v_doc> / (max(||v_claim||,eps) * max(||v_doc||,eps))
    div   = 1 - align ; tens = div^2
    h      = relu([h_final | align | div | tens] @ W1 + b1)
    shared = relu(h @ W2 + b2)
    out    = sigmoid(shared @ Wr + br)

Strategy: data-parallel over batch on 8 cores (2048 rows/core). The whole
MLP chain runs in transposed space (features on partitions, batch on the
free dim) so W1/W2/Wr load from DRAM directly as the stationary (lhsT)
matmul operand with no weight transpose. Only h_final needs a physical
transpose, done on the PE with an identity matrix, in bf16. All DRAM loads
go through HWDGE (nc.sync) as f32 — SWDGE cast-DMAs hit the walrus
"too many sync wait commands" limit — and the bf16 casts run on the
otherwise-idle GpSimd engine. Cosine stats accumulate in f32 (DVE
tensor_tensor_reduce + ACT Square-accumulate), the [align,div,tens] extras
enter stage 1 as 3 extra contraction rows (a K=128 matmul against a
zero-padded lhsT block), and the biases ride the Relu/Sigmoid activations.
Stage 1 is kt-outer over 4 PSUM accumulators so its matmuls track the W1
DMA stream during the prologue; each next-superchunk's input prep is
interleaved between stage-1 quarters to keep the PE transposes spread out.

Stages 1 and 2 run in fp8e4m3 DoubleRow mode (2 contraction rows per PE
cell per cycle = 2x bf16 matmul throughput). Weights are scaled by 64 at
the f32->fp8 cast so W~N(0,1/d) values clear the fp8 denormal floor; the
scale comes back out via the Relu activation's scale=1/64 pre-multiply.
The 3-row extras block stays bf16 (accumulates into the same PSUM group),
and stage 3 + all stats stay bf16/f32, so only the two big GEMMs pay the
fp8 rounding (~3% per element, averaged down by the 2048-term sums).
"""

import numpy as np

P = 128
D = 2048
NCORES = 8
FREE = 512          # moving free dim / batch-chunk width
KT = D // P         # 16 k-tiles for stage 1 contraction
NT = D // P         # 16 n-tiles  (stage-1 output features)
J = D // 2          # 1024
JT = J // P         # 8 j-tiles  (stage-2 output features)
HD = D // 2         # stats half width
EPS = 1e-12
WS = 64.0           # fp8 weight pre-scale (undone by activation scale=1/WS)

_cache = {}


def _build(bc, reps=1):
    """Build the per-core Bass program for bc batch rows.

    reps > 1 repeats the whole pipeline over the same inputs inside one
    NEFF — used only for timing (amortizes host dispatch overhead).
    """
    import concourse.bass as bass
    import concourse.tile as tile
    from concourse import bacc, mybir
    from concourse.masks import make_identity

    f32 = mybir.dt.float32
    bf16 = mybir.dt.bfloat16
    AF = mybir.ActivationFunctionType
    OP = mybir.AluOpType

    nsc = bc // FREE            # super-chunks (= batch chunks) per core
    nmt = FREE // P             # m-tiles per super-chunk (4)

    nc = bacc.Bacc(trn_type="TRN2", target_bir_lowering=False, debug=False)

    h_final = nc.dram_tensor("h_final", [bc, D], f32, kind="ExternalInput").ap()
    v_claim = nc.dram_tensor("v_claim", [bc, D], f32, kind="ExternalInput").ap()
    v_doc = nc.dram_tensor("v_doc", [bc, D], f32, kind="ExternalInput").ap()
    W1 = nc.dram_tensor("W1", [D + 3, D], f32, kind="ExternalInput").ap()
    b1 = nc.dram_tensor("b1", [D], f32, kind="ExternalInput").ap()
    W2 = nc.dram_tensor("W2", [D, J], f32, kind="ExternalInput").ap()
    b2 = nc.dram_tensor("b2", [J], f32, kind="ExternalInput").ap()
    Wr = nc.dram_tensor("Wr", [J, 1], f32, kind="ExternalInput").ap()
    br = nc.dram_tensor("br", [1], f32, kind="ExternalInput").ap()
    out = nc.dram_tensor("out", [bc, 1], f32, kind="ExternalOutput").ap()

    with tile.TileContext(nc) as tc:
        with (
            tc.tile_pool(name="singles", bufs=1) as singles,
            tc.tile_pool(name="xt", bufs=2) as xt_pool,
            tc.tile_pool(name="ht", bufs=1) as ht_pool,
            tc.tile_pool(name="st", bufs=1) as st_pool,
            tc.tile_pool(name="stage", bufs=2) as stage,
            tc.tile_pool(name="stats", bufs=2) as stats,
            tc.tile_pool(name="psA", bufs=1, space="PSUM") as psA,
            tc.tile_pool(name="psB", bufs=2, space="PSUM") as psB,
            tc.tile_pool(name="psT", bufs=2, space="PSUM") as psT,
        ):
            # ---- constants; the strided small DMAs (b1/b2/Wr: thousands
            # of 4-byte descriptors) are deferred until after sc0's x loads
            # so they don't block the HWDGE FIFO at kernel start ----
            ident = singles.tile([P, P], bf16)
            make_identity(nc, ident)
            identf = singles.tile([P, P], f32)
            make_identity(nc, identf)
            b1sb = singles.tile([P, NT], f32)
            b2sb = singles.tile([P, JT], f32)
            wrf = singles.tile([P, JT], f32)
            wrsb = singles.tile([P, JT], bf16)
            brsb = singles.tile([1, 1], f32)
            ex1sb = singles.tile([P, D], bf16)
            nc.vector.memset(ex1sb, 0.0)
            featsT = singles.tile([P, bc], bf16)
            nc.vector.memset(featsT, 0.0)

            def load_via_transpose(dst, src_1d, n, nm):
                # contiguous [n, 128] load + PE transpose instead of a
                # 4-byte-strided DMA (n*128 descriptors -> n descriptors)
                t = stats.tile([P, P], f32, tag="cst", name=f"cst{nm}", bufs=2)
                nc.vector.memset(t, 0.0)
                nc.sync.dma_start(t[0:n, :], src_1d.rearrange("(o p) -> o p", p=P))
                pst = psT.tile([P, P], f32, tag="tp", name=f"cstp{nm}")
                nc.tensor.transpose(pst, t, identf)
                nc.vector.tensor_copy(dst, pst[:, 0:n])

            def load_small_consts():
                load_via_transpose(b1sb, b1, NT, "b1")
                load_via_transpose(b2sb, b2, JT, "b2")
                load_via_transpose(wrf, Wr.rearrange("k one -> (k one)"), JT, "wr")
                nc.gpsimd.tensor_copy(wrsb, wrf)
                nc.sync.dma_start(brsb, br[None, :])
                for qc in range(nmt):
                    cols = slice(qc * FREE, (qc + 1) * FREE)
                    exq = stage.tile([P, FREE], f32, tag="w1q", name=f"exf{qc}",
                                     bufs=3)
                    nc.sync.dma_start(exq[0:3, :], W1[D:D + 3, cols])
                    nc.gpsimd.tensor_copy(ex1sb[0:3, cols], exq[0:3, :])
            # big weights declared here, streamed + cast after sc0's x loads
            w1sb = singles.tile([P, KT, D], bf16)
            w2sb = singles.tile([P, KT, J], bf16)

            def cast_copy(i, out_ap, in_ap):
                # spread the f32->bf16 weight casts across three engines so
                # the staging slots recycle fast enough to keep DMA streaming
                eng = i % 3
                if eng == 0:
                    nc.gpsimd.tensor_copy(out_ap, in_ap)
                elif eng == 1:
                    nc.vector.tensor_copy(out_ap, in_ap)
                else:
                    nc.scalar.activation(out_ap, in_ap, AF.Copy)

            def load_w1(kt):
                wf = stage.tile([P, D], f32, tag="f32s", name=f"w1f{kt}", bufs=2)
                nc.sync.dma_start(wf, W1[kt * P:(kt + 1) * P, :])
                cast_copy(kt, w1sb[:, kt, :], wf)

            def load_w1_q(kt, qc):
                # column-quarter load: stage-1 quarter qc only reads
                # w1sb[:, kt, qc*512:(qc+1)*512], so streaming W1 in
                # quarter-column order unblocks each stage-1 quarter after
                # ~4.2 MB instead of the full 16.8 MB
                cols = slice(qc * FREE, (qc + 1) * FREE)
                wf = stage.tile([P, FREE], f32, tag="w1q", name=f"w1q{kt}_{qc}",
                                bufs=3)
                nc.sync.dma_start(wf, W1[kt * P:(kt + 1) * P, cols])
                cast_copy(kt + qc, w1sb[:, kt, cols], wf)

            def load_w2_h(kt, ch):
                # column-half load: stage-2 jt-chains 0-3 only read
                # w2sb[:, :, 0:512], so streaming W2 in column-half order
                # lets stage 2 of sc0 start after half the W2 bytes
                cols = slice(ch * FREE, (ch + 1) * FREE)
                wf = stage.tile([P, FREE], f32, tag="w1q", name=f"w2h{kt}_{ch}",
                                bufs=3)
                nc.sync.dma_start(wf, W2[kt * P:(kt + 1) * P, cols])
                cast_copy(kt + ch, w2sb[:, kt, cols], wf)

            sc_state = {}

            def rowbase(sc):
                return (sc % nsc) * nmt

            def phaseA_start(sc):
                sc_state[sc] = dict(
                    ccs=stats.tile([P, nmt], f32, tag="ccs", name=f"ccs{sc}"),
                    dds=stats.tile([P, nmt], f32, tag="dds", name=f"dds{sc}"),
                    cds=stats.tile([P, nmt], f32, tag="cds", name=f"cds{sc}"),
                    xt=xt_pool.tile([P, KT, FREE], bf16, tag="xt", name=f"xt{sc}"),
                )

            def phaseA_x(sc, mt):
                # bf16 cast on the (idle) GpSimd engine, then 1-cyc/row bf16
                # PE transposes; the DVE psum copy moves them into xt
                s = sc_state[sc]
                row = (rowbase(sc) + mt) * P
                xf = stage.tile([P, D], f32, tag="xf32", name=f"xf{sc}_{mt}", bufs=2)
                nc.sync.dma_start(xf, h_final[row:row + P, :])
                xbf = stage.tile([P, D], bf16, tag="xbf", name=f"xbf{sc}_{mt}",
                                 bufs=2)
                nc.gpsimd.tensor_copy(xbf, xf)
                for kt in range(KT):
                    pst = psT.tile([P, P], bf16, tag="tp", name=f"tp{sc}_{mt}_{kt}")
                    nc.tensor.transpose(pst, xbf[:, kt * P:(kt + 1) * P], ident)
                    nc.vector.tensor_copy(s["xt"][:, kt, mt * P:(mt + 1) * P], pst)

            def phaseA_v(sc, mt):
                # cosine stats for one m-tile, in f32 halves
                s = sc_state[sc]
                row = (rowbase(sc) + mt) * P
                nq = 4
                hsum = stats.tile([P, 3, nq], f32, tag="hsum", name=f"hs{sc}_{mt}")
                for h in range(nq):
                    QW = D // nq
                    cols = slice(h * QW, (h + 1) * QW)
                    vcf = stage.tile([P, QW], f32, tag="vcf", name=f"vc{sc}_{mt}{h}")
                    nc.sync.dma_start(vcf, v_claim[row:row + P, cols])
                    vdf = stage.tile([P, QW], f32, tag="vdf", name=f"vd{sc}_{mt}{h}")
                    nc.sync.dma_start(vdf, v_doc[row:row + P, cols])
                    # NOTE: tensor_tensor_reduce crashes TRN2 here (device
                    # unrecoverable) — use mult + reduce_sum instead
                    trash = stage.tile([P, QW], bf16, tag="trash",
                                       name=f"tr{sc}_{mt}{h}")
                    nc.vector.tensor_mul(trash, vcf, vdf)
                    nc.vector.reduce_sum(hsum[:, 0, h:h + 1], trash,
                                         axis=mybir.AxisListType.X)
                    # in-place squares (after the DVE read above)
                    nc.scalar.activation(vcf, vcf, AF.Square,
                                         accum_out=hsum[:, 1, h:h + 1])
                    nc.scalar.activation(vdf, vdf, AF.Square,
                                         accum_out=hsum[:, 2, h:h + 1])
                nc.vector.reduce_sum(s["cds"][:, mt:mt + 1], hsum[:, 0, :],
                                     axis=mybir.AxisListType.X)
                nc.vector.reduce_sum(s["ccs"][:, mt:mt + 1], hsum[:, 1, :],
                                     axis=mybir.AxisListType.X)
                nc.vector.reduce_sum(s["dds"][:, mt:mt + 1], hsum[:, 2, :],
                                     axis=mybir.AxisListType.X)

            def phaseA_mtile(sc, mt):
                phaseA_x(sc, mt)
                phaseA_v(sc, mt)

            def phaseA_finish(sc):
                # stats -> [align, div, tens] rows of featsT
                s = sc_state[sc]
                ccs, dds, cds = s["ccs"], s["dds"], s["cds"]
                feats = stats.tile([P, nmt, 3], f32, tag="feats", name=f"ft{sc}")
                featsb = stats.tile([P, nmt, 3], bf16, tag="featsb", name=f"fb{sc}")
                nc.scalar.activation(ccs, ccs, AF.Sqrt)
                nc.scalar.activation(dds, dds, AF.Sqrt)
                nc.vector.tensor_scalar_max(ccs, ccs, EPS)
                nc.vector.tensor_scalar_max(dds, dds, EPS)
                nc.vector.tensor_mul(ccs, ccs, dds)
                nc.vector.reciprocal(ccs, ccs)
                nc.vector.tensor_mul(feats[:, :, 0], cds, ccs)      # align
                nc.vector.tensor_scalar(feats[:, :, 1], feats[:, :, 0],
                                        -1.0, 1.0, OP.mult, OP.add)  # div
                nc.vector.tensor_mul(feats[:, :, 2], feats[:, :, 1],
                                     feats[:, :, 1])                 # tens
                nc.vector.tensor_copy(featsb, feats)
                for mt in range(nmt):
                    # per-m-tile transpose: engine reads of PSUM must start at
                    # a 32-aligned partition, so each [3, P] block gets its own
                    # psum tile based at partition 0
                    psf = psT.tile([3, P], bf16, tag="tp", name=f"psf{sc}_{mt}")
                    nc.tensor.transpose(psf, featsb[:, mt, :], ident)
                    col = (rowbase(sc) + mt) * P
                    nc.vector.tensor_copy(featsT[0:3, col:col + P], psf)

            # prologue: sc0 x tiles first (PE starts transposing after ~1 MB),
            # then W1 in column-quarter order (all kt of quarter 0 first, so
            # stage-1 quarter q unblocks after (q+1)*4.2 MB), with the
            # v_claim/v_doc loads interleaved, then W2
            phaseA_start(0)
            for mt in range(nmt):
                phaseA_x(0, mt)
            load_small_consts()
            for g in range(nmt):
                for kt in range(KT):
                    load_w1_q(kt, g)
                phaseA_v(0, g)
            for ch in range(2):
                for kt in range(KT):
                    load_w2_h(kt, ch)
            phaseA_finish(0)

            total_sc = nsc * reps
            for sc in range(total_sc):
                nxt = sc + 1 if sc + 1 < total_sc else None
                if nxt is not None:
                    phaseA_start(nxt)
                mcols = slice((sc % nsc) * FREE, (sc % nsc + 1) * FREE)

                # ---- stage 1: hT[n, m] = relu(W1.T @ xT + extras + b1) ----
                # kt-outer over 4 psum accumulators: on sc0 the matmuls track
                # the W1 DMA stream k-slice by k-slice instead of stalling on
                # the full 16.8 MB
                ht = ht_pool.tile([P, NT, FREE], bf16)
                xt = sc_state[sc]["xt"]
                NACC = 4
                for q in range(NT // NACC):
                    pss = [psA.tile([P, FREE], mybir.dt.float32, tag=f"ps1_{i}",
                                    name=f"ps1_{sc}_{q}_{i}")
                           for i in range(NACC)]
                    for kt in range(KT):
                        for i in range(NACC):
                            nt = q * NACC + i
                            nc.tensor.matmul(pss[i],
                                             w1sb[:, kt, nt * P:(nt + 1) * P],
                                             xt[:, kt, :], start=(kt == 0),
                                             stop=False)
                    for i in range(NACC):
                        nt = q * NACC + i
                        nc.tensor.matmul(pss[i], ex1sb[:, nt * P:(nt + 1) * P],
                                         featsT[:, mcols], start=False, stop=True)
                        nc.scalar.activation(ht[:, nt, :], pss[i], AF.Relu,
                                             bias=b1sb[:, nt:nt + 1])
                    # interleave next-sc input prep between stage-1 quarters
                    # so its PE transposes spread out instead of bunching
                    if nxt is not None:
                        phaseA_mtile(nxt, q)

                if nxt is not None:
                    phaseA_finish(nxt)

                # ---- stage 2: sT[j, m] = relu(W2.T @ hT + b2) ----
                st = st_pool.tile([P, JT, FREE], bf16)
                for jt in range(JT):
                    ps = psB.tile([P, FREE], mybir.dt.float32, tag="ps2")
                    for nt in range(NT):
                        nc.tensor.matmul(ps, w2sb[:, nt, jt * P:(jt + 1) * P],
                                         ht[:, nt, :], start=(nt == 0),
                                         stop=(nt == NT - 1))
                    nc.scalar.activation(st[:, jt, :], ps, AF.Relu,
                                         bias=b2sb[:, jt:jt + 1])

                # ---- stage 3: out[m] = sigmoid(Wr.T @ sT + br) ----
                psd = psB.tile([1, FREE], mybir.dt.float32, tag="ps2")
                for jt in range(JT):
                    nc.tensor.matmul(psd, wrsb[:, jt:jt + 1], st[:, jt, :],
                                     start=(jt == 0), stop=(jt == JT - 1))
                osb = stats.tile([1, FREE], f32, tag="osb", name=f"osb{sc}")
                nc.scalar.activation(osb, psd, AF.Sigmoid, bias=brsb[0:1, 0:1])
                nc.sync.dma_start(
                    out.rearrange("m one -> one m")[:, mcols], osb)

    nc.compile()
    return nc


def get_nc(bc, reps=1):
    if (bc, reps) not in _cache:
        _cache[(bc, reps)] = _build(bc, reps)
    return _cache[(bc, reps)]


def _shim_axon_hooks():
    """antenv.axon_hooks is absent in this container; shim it so a
    BASS_TRACE=1 environment can't crash run_bass_kernel_spmd."""
    import sys
    import types
    try:
        import antenv
    except ImportError:
        return
    if "antenv.axon_hooks" not in sys.modules:
        try:
            import antenv.axon_hooks  # noqa: F401
        except ImportError:
            m = types.ModuleType("antenv.axon_hooks")
            m.get_axon_ntff_profile_hook = lambda: None
            sys.modules["antenv.axon_hooks"] = m
            antenv.axon_hooks = m


def kernel(**inputs):
    _shim_axon_hooks()
    from concourse.bass_utils import run_bass_kernel_spmd

    B = inputs["h_final"].shape[0]
    bc = B // NCORES
    nc = get_nc(bc)
    shard_keys = ("h_final", "v_claim", "v_doc")
    in_maps = []
    for c in range(NCORES):
        m = {}
        for k, v in inputs.items():
            v = np.asarray(v)
            if k in shard_keys:
                v = v[c * bc:(c + 1) * bc]
            m[k] = np.ascontiguousarray(v)
        in_maps.append(m)
    res = run_bass_kernel_spmd(nc, in_maps, core_ids=list(range(NCORES)))
    return np.concatenate([r["out"] for r in res.results], axis=0)



# revision 9
# speedup vs baseline: 1486.5493x; 1486.5493x over previous
"""Trainium2 Bass kernel for nn_DocumentHead (retrieval head MLP).

Math (per batch row):
    align = <v_claim, v_doc> / (max(||v_claim||,eps) * max(||v_doc||,eps))
    div   = 1 - align ; tens = div^2
    h      = relu([h_final | align | div | tens] @ W1 + b1)
    shared = relu(h @ W2 + b2)
    out    = sigmoid(shared @ Wr + br)

Strategy: data-parallel over batch on 8 cores (2048 rows/core). The whole
MLP chain runs in transposed space (features on partitions, batch on the
free dim) so W1/W2/Wr load from DRAM directly as the stationary (lhsT)
matmul operand with no weight transpose. Only h_final needs a physical
transpose, done on the PE with an identity matrix, in bf16. All DRAM loads
go through HWDGE (nc.sync) as f32 — SWDGE cast-DMAs hit the walrus
"too many sync wait commands" limit — and the bf16 casts run on the
otherwise-idle GpSimd engine. Cosine stats accumulate in f32 (DVE
tensor_tensor_reduce + ACT Square-accumulate), the [align,div,tens] extras
enter stage 1 as 3 extra contraction rows (a K=128 matmul against a
zero-padded lhsT block), and the biases ride the Relu/Sigmoid activations.
Stage 1 is kt-outer over 4 PSUM accumulators so its matmuls track the W1
DMA stream during the prologue; each next-superchunk's input prep is
interleaved between stage-1 quarters to keep the PE transposes spread out.
"""

import numpy as np

P = 128
D = 2048
NCORES = 8
FREE = 512          # moving free dim / batch-chunk width
KT = D // P         # 16 k-tiles for stage 1 contraction
NT = D // P         # 16 n-tiles  (stage-1 output features)
J = D // 2          # 1024
JT = J // P         # 8 j-tiles  (stage-2 output features)
HD = D // 2         # stats half width
EPS = 1e-12

_cache = {}


def _build(bc, reps=1):
    """Build the per-core Bass program for bc batch rows.

    reps > 1 repeats the whole pipeline over the same inputs inside one
    NEFF — used only for timing (amortizes host dispatch overhead).
    """
    import concourse.bass as bass
    import concourse.tile as tile
    from concourse import bacc, mybir
    from concourse.masks import make_identity

    f32 = mybir.dt.float32
    bf16 = mybir.dt.bfloat16
    AF = mybir.ActivationFunctionType
    OP = mybir.AluOpType

    nsc = bc // FREE            # super-chunks (= batch chunks) per core
    nmt = FREE // P             # m-tiles per super-chunk (4)

    nc = bacc.Bacc(trn_type="TRN2", target_bir_lowering=False, debug=False)

    h_final = nc.dram_tensor("h_final", [bc, D], f32, kind="ExternalInput").ap()
    v_claim = nc.dram_tensor("v_claim", [bc, D], f32, kind="ExternalInput").ap()
    v_doc = nc.dram_tensor("v_doc", [bc, D], f32, kind="ExternalInput").ap()
    W1 = nc.dram_tensor("W1", [D + 3, D], f32, kind="ExternalInput").ap()
    b1 = nc.dram_tensor("b1", [D], f32, kind="ExternalInput").ap()
    W2 = nc.dram_tensor("W2", [D, J], f32, kind="ExternalInput").ap()
    b2 = nc.dram_tensor("b2", [J], f32, kind="ExternalInput").ap()
    Wr = nc.dram_tensor("Wr", [J, 1], f32, kind="ExternalInput").ap()
    br = nc.dram_tensor("br", [1], f32, kind="ExternalInput").ap()
    out = nc.dram_tensor("out", [bc, 1], f32, kind="ExternalOutput").ap()

    with tile.TileContext(nc) as tc:
        with (
            tc.tile_pool(name="singles", bufs=1) as singles,
            tc.tile_pool(name="xt", bufs=2) as xt_pool,
            tc.tile_pool(name="ht", bufs=1) as ht_pool,
            tc.tile_pool(name="st", bufs=1) as st_pool,
            tc.tile_pool(name="stage", bufs=2) as stage,
            tc.tile_pool(name="stats", bufs=2) as stats,
            tc.tile_pool(name="psA", bufs=1, space="PSUM") as psA,
            tc.tile_pool(name="psB", bufs=2, space="PSUM") as psB,
            tc.tile_pool(name="psT", bufs=2, space="PSUM") as psT,
        ):
            # ---- constants; the strided small DMAs (b1/b2/Wr: thousands
            # of 4-byte descriptors) are deferred until after sc0's x loads
            # so they don't block the HWDGE FIFO at kernel start ----
            ident = singles.tile([P, P], bf16)
            make_identity(nc, ident)
            identf = singles.tile([P, P], f32)
            make_identity(nc, identf)
            b1sb = singles.tile([P, NT], f32)
            b2sb = singles.tile([P, JT], f32)
            wrf = singles.tile([P, JT], f32)
            wrsb = singles.tile([P, JT], bf16)
            brsb = singles.tile([1, 1], f32)
            ex1sb = singles.tile([P, D], bf16)
            nc.vector.memset(ex1sb, 0.0)
            featsT = singles.tile([P, bc], bf16)
            nc.vector.memset(featsT, 0.0)

            def load_via_transpose(dst, src_1d, n, nm):
                # contiguous [n, 128] load + PE transpose instead of a
                # 4-byte-strided DMA (n*128 descriptors -> n descriptors)
                t = stats.tile([P, P], f32, tag="cst", name=f"cst{nm}", bufs=2)
                nc.vector.memset(t, 0.0)
                nc.sync.dma_start(t[0:n, :], src_1d.rearrange("(o p) -> o p", p=P))
                pst = psT.tile([P, P], f32, tag="tp", name=f"cstp{nm}")
                nc.tensor.transpose(pst, t, identf)
                nc.vector.tensor_copy(dst, pst[:, 0:n])

            def load_small_consts():
                load_via_transpose(b1sb, b1, NT, "b1")
                load_via_transpose(b2sb, b2, JT, "b2")
                load_via_transpose(wrf, Wr.rearrange("k one -> (k one)"), JT, "wr")
                nc.gpsimd.tensor_copy(wrsb, wrf)
                nc.sync.dma_start(brsb, br[None, :])
                for qc in range(nmt):
                    cols = slice(qc * FREE, (qc + 1) * FREE)
                    exq = stage.tile([P, FREE], f32, tag="w1q", name=f"exf{qc}",
                                     bufs=3)
                    nc.sync.dma_start(exq[0:3, :], W1[D:D + 3, cols])
                    nc.gpsimd.tensor_copy(ex1sb[0:3, cols], exq[0:3, :])
            # big weights declared here, streamed + cast after sc0's x loads
            w1sb = singles.tile([P, KT, D], bf16)
            w2sb = singles.tile([P, KT, J], bf16)

            def cast_copy(i, out_ap, in_ap):
                # spread the f32->bf16 weight casts across three engines so
                # the staging slots recycle fast enough to keep DMA streaming
                eng = i % 3
                if eng == 0:
                    nc.gpsimd.tensor_copy(out_ap, in_ap)
                elif eng == 1:
                    nc.vector.tensor_copy(out_ap, in_ap)
                else:
                    nc.scalar.activation(out_ap, in_ap, AF.Copy)

            def load_w1(kt):
                wf = stage.tile([P, D], f32, tag="f32s", name=f"w1f{kt}", bufs=2)
                nc.sync.dma_start(wf, W1[kt * P:(kt + 1) * P, :])
                cast_copy(kt, w1sb[:, kt, :], wf)

            def load_w1_q(kt, qc):
                # column-quarter load: stage-1 quarter qc only reads
                # w1sb[:, kt, qc*512:(qc+1)*512], so streaming W1 in
                # quarter-column order unblocks each stage-1 quarter after
                # ~4.2 MB instead of the full 16.8 MB
                cols = slice(qc * FREE, (qc + 1) * FREE)
                wf = stage.tile([P, FREE], f32, tag="w1q", name=f"w1q{kt}_{qc}",
                                bufs=3)
                nc.sync.dma_start(wf, W1[kt * P:(kt + 1) * P, cols])
                cast_copy(kt + qc, w1sb[:, kt, cols], wf)

            def load_w2_h(kt, ch):
                # column-half load: stage-2 jt-chains 0-3 only read
                # w2sb[:, :, 0:512], so streaming W2 in column-half order
                # lets stage 2 of sc0 start after half the W2 bytes
                cols = slice(ch * FREE, (ch + 1) * FREE)
                wf = stage.tile([P, FREE], f32, tag="w1q", name=f"w2h{kt}_{ch}",
                                bufs=3)
                nc.sync.dma_start(wf, W2[kt * P:(kt + 1) * P, cols])
                cast_copy(kt + ch, w2sb[:, kt, cols], wf)

            sc_state = {}

            def rowbase(sc):
                return (sc % nsc) * nmt

            def phaseA_start(sc):
                sc_state[sc] = dict(
                    ccs=stats.tile([P, nmt], f32, tag="ccs", name=f"ccs{sc}"),
                    dds=stats.tile([P, nmt], f32, tag="dds", name=f"dds{sc}"),
                    cds=stats.tile([P, nmt], f32, tag="cds", name=f"cds{sc}"),
                    xt=xt_pool.tile([P, KT, FREE], bf16, tag="xt", name=f"xt{sc}"),
                )

            def phaseA_x(sc, mt):
                # bf16 cast on the (idle) GpSimd engine, then 1-cyc/row bf16
                # PE transposes; the DVE psum copy moves them into xt
                s = sc_state[sc]
                row = (rowbase(sc) + mt) * P
                xf = stage.tile([P, D], f32, tag="xf32", name=f"xf{sc}_{mt}", bufs=2)
                nc.sync.dma_start(xf, h_final[row:row + P, :])
                xbf = stage.tile([P, D], bf16, tag="xbf", name=f"xbf{sc}_{mt}",
                                 bufs=2)
                nc.gpsimd.tensor_copy(xbf, xf)
                for kt in range(KT):
                    pst = psT.tile([P, P], bf16, tag="tp", name=f"tp{sc}_{mt}_{kt}")
                    nc.tensor.transpose(pst, xbf[:, kt * P:(kt + 1) * P], ident)
                    nc.vector.tensor_copy(s["xt"][:, kt, mt * P:(mt + 1) * P], pst)

            def phaseA_v(sc, mt):
                # cosine stats for one m-tile, in f32 halves
                s = sc_state[sc]
                row = (rowbase(sc) + mt) * P
                nq = 4
                hsum = stats.tile([P, 3, nq], f32, tag="hsum", name=f"hs{sc}_{mt}")
                for h in range(nq):
                    QW = D // nq
                    cols = slice(h * QW, (h + 1) * QW)
                    vcf = stage.tile([P, QW], f32, tag="vcf", name=f"vc{sc}_{mt}{h}")
                    nc.sync.dma_start(vcf, v_claim[row:row + P, cols])
                    vdf = stage.tile([P, QW], f32, tag="vdf", name=f"vd{sc}_{mt}{h}")
                    nc.sync.dma_start(vdf, v_doc[row:row + P, cols])
                    # NOTE: tensor_tensor_reduce crashes TRN2 here (device
                    # unrecoverable) — use mult + reduce_sum instead
                    trash = stage.tile([P, QW], bf16, tag="trash",
                                       name=f"tr{sc}_{mt}{h}")
                    nc.vector.tensor_mul(trash, vcf, vdf)
                    nc.vector.reduce_sum(hsum[:, 0, h:h + 1], trash,
                                         axis=mybir.AxisListType.X)
                    # in-place squares (after the DVE read above)
                    nc.scalar.activation(vcf, vcf, AF.Square,
                                         accum_out=hsum[:, 1, h:h + 1])
                    nc.scalar.activation(vdf, vdf, AF.Square,
                                         accum_out=hsum[:, 2, h:h + 1])
                nc.vector.reduce_sum(s["cds"][:, mt:mt + 1], hsum[:, 0, :],
                                     axis=mybir.AxisListType.X)
                nc.vector.reduce_sum(s["ccs"][:, mt:mt + 1], hsum[:, 1, :],
                                     axis=mybir.AxisListType.X)
                nc.vector.reduce_sum(s["dds"][:, mt:mt + 1], hsum[:, 2, :],
                                     axis=mybir.AxisListType.X)

            def phaseA_mtile(sc, mt):
                phaseA_x(sc, mt)
                phaseA_v(sc, mt)

            def phaseA_finish(sc):
                # stats -> [align, div, tens] rows of featsT
                s = sc_state[sc]
                ccs, dds, cds = s["ccs"], s["dds"], s["cds"]
                feats = stats.tile([P, nmt, 3], f32, tag="feats", name=f"ft{sc}")
                featsb = stats.tile([P, nmt, 3], bf16, tag="featsb", name=f"fb{sc}")
                nc.scalar.activation(ccs, ccs, AF.Sqrt)
                nc.scalar.activation(dds, dds, AF.Sqrt)
                nc.vector.tensor_scalar_max(ccs, ccs, EPS)
                nc.vector.tensor_scalar_max(dds, dds, EPS)
                nc.vector.tensor_mul(ccs, ccs, dds)
                nc.vector.reciprocal(ccs, ccs)
                nc.vector.tensor_mul(feats[:, :, 0], cds, ccs)      # align
                nc.vector.tensor_scalar(feats[:, :, 1], feats[:, :, 0],
                                        -1.0, 1.0, OP.mult, OP.add)  # div
                nc.vector.tensor_mul(feats[:, :, 2], feats[:, :, 1],
                                     feats[:, :, 1])                 # tens
                nc.vector.tensor_copy(featsb, feats)
                for mt in range(nmt):
                    # per-m-tile transpose: engine reads of PSUM must start at
                    # a 32-aligned partition, so each [3, P] block gets its own
                    # psum tile based at partition 0
                    psf = psT.tile([3, P], bf16, tag="tp", name=f"psf{sc}_{mt}")
                    nc.tensor.transpose(psf, featsb[:, mt, :], ident)
                    col = (rowbase(sc) + mt) * P
                    nc.vector.tensor_copy(featsT[0:3, col:col + P], psf)

            # prologue: sc0 x tiles first (PE starts transposing after ~1 MB),
            # then W1 in column-quarter order (all kt of quarter 0 first, so
            # stage-1 quarter q unblocks after (q+1)*4.2 MB), with the
            # v_claim/v_doc loads interleaved, then W2
            phaseA_start(0)
            for mt in range(nmt):
                phaseA_x(0, mt)
            load_small_consts()
            for g in range(nmt):
                for kt in range(KT):
                    load_w1_q(kt, g)
                phaseA_v(0, g)
            for ch in range(2):
                for kt in range(KT):
                    load_w2_h(kt, ch)
            phaseA_finish(0)

            total_sc = nsc * reps
            for sc in range(total_sc):
                nxt = sc + 1 if sc + 1 < total_sc else None
                if nxt is not None:
                    phaseA_start(nxt)
                mcols = slice((sc % nsc) * FREE, (sc % nsc + 1) * FREE)

                # ---- stage 1: hT[n, m] = relu(W1.T @ xT + extras + b1) ----
                # kt-outer over 4 psum accumulators: on sc0 the matmuls track
                # the W1 DMA stream k-slice by k-slice instead of stalling on
                # the full 16.8 MB
                ht = ht_pool.tile([P, NT, FREE], bf16)
                xt = sc_state[sc]["xt"]
                NACC = 4
                for q in range(NT // NACC):
                    pss = [psA.tile([P, FREE], mybir.dt.float32, tag=f"ps1_{i}",
                                    name=f"ps1_{sc}_{q}_{i}")
                           for i in range(NACC)]
                    for kt in range(KT):
                        for i in range(NACC):
                            nt = q * NACC + i
                            nc.tensor.matmul(pss[i],
                                             w1sb[:, kt, nt * P:(nt + 1) * P],
                                             xt[:, kt, :], start=(kt == 0),
                                             stop=False)
                    for i in range(NACC):
                        nt = q * NACC + i
                        nc.tensor.matmul(pss[i], ex1sb[:, nt * P:(nt + 1) * P],
                                         featsT[:, mcols], start=False, stop=True)
                        nc.scalar.activation(ht[:, nt, :], pss[i], AF.Relu,
                                             bias=b1sb[:, nt:nt + 1])
                    # interleave next-sc input prep between stage-1 quarters
                    # so its PE transposes spread out instead of bunching
                    if nxt is not None:
                        phaseA_mtile(nxt, q)

                if nxt is not None:
                    phaseA_finish(nxt)

                # ---- stage 2: sT[j, m] = relu(W2.T @ hT + b2) ----
                st = st_pool.tile([P, JT, FREE], bf16)
                for jt in range(JT):
                    ps = psB.tile([P, FREE], mybir.dt.float32, tag="ps2")
                    for nt in range(NT):
                        nc.tensor.matmul(ps, w2sb[:, nt, jt * P:(jt + 1) * P],
                                         ht[:, nt, :], start=(nt == 0),
                                         stop=(nt == NT - 1))
                    nc.scalar.activation(st[:, jt, :], ps, AF.Relu,
                                         bias=b2sb[:, jt:jt + 1])

                # ---- stage 3: out[m] = sigmoid(Wr.T @ sT + br) ----
                psd = psB.tile([1, FREE], mybir.dt.float32, tag="ps2")
                for jt in range(JT):
                    nc.tensor.matmul(psd, wrsb[:, jt:jt + 1], st[:, jt, :],
                                     start=(jt == 0), stop=(jt == JT - 1))
                osb = stats.tile([1, FREE], f32, tag="osb", name=f"osb{sc}")
                nc.scalar.activation(osb, psd, AF.Sigmoid, bias=brsb[0:1, 0:1])
                nc.sync.dma_start(
                    out.rearrange("m one -> one m")[:, mcols], osb)

    nc.compile()
    return nc


def get_nc(bc, reps=1):
    if (bc, reps) not in _cache:
        _cache[(bc, reps)] = _build(bc, reps)
    return _cache[(bc, reps)]


def _shim_axon_hooks():
    """antenv.axon_hooks is absent in this container; shim it so a
    BASS_TRACE=1 environment can't crash run_bass_kernel_spmd."""
    import sys
    import types
    try:
        import antenv
    except ImportError:
        return
    if "antenv.axon_hooks" not in sys.modules:
        try:
            import antenv.axon_hooks  # noqa: F401
        except ImportError:
            m = types.ModuleType("antenv.axon_hooks")
            m.get_axon_ntff_profile_hook = lambda: None
            sys.modules["antenv.axon_hooks"] = m
            antenv.axon_hooks = m


def kernel(**inputs):
    _shim_axon_hooks()
    from concourse.bass_utils import run_bass_kernel_spmd

    B = inputs["h_final"].shape[0]
    bc = B // NCORES
    nc = get_nc(bc)
    shard_keys = ("h_final", "v_claim", "v_doc")
    in_maps = []
    for c in range(NCORES):
        m = {}
        for k, v in inputs.items():
            v = np.asarray(v)
            if k in shard_keys:
                v = v[c * bc:(c + 1) * bc]
            m[k] = np.ascontiguousarray(v)
        in_maps.append(m)
    res = run_bass_kernel_spmd(nc, in_maps, core_ids=list(range(NCORES)))
    return np.concatenate([r["out"] for r in res.results], axis=0)

